# revision 11
# baseline (speedup 1.0000x reference)
"""Soft decision-tree layer (depth 4, 16 leaves) on 8 trn2 NeuronCores.

Sharding: 2-way data parallel (token halves) x 4-way expert parallel
(4 leaves per core).  Each core computes, for its 2048 tokens t and its
4 leaves l:  partial[t,:] = sum_l path_l(t) * (x[t] @ Wl[l]), plus it
exports the path probabilities; the host adds the path-weighted leaf
biases (sum_l path_l * bl[l]) in float64 and sums the 4 expert partials
per token half.

GEMM operands are float16 (streams at the full 1 col/cycle PE rate with
the weight load pipelined under the previous matmul; ~216ns per
128x128x512 matmul); accumulation is fp32 in PSUM/SBUF; partials are
written back as bf16 (halves the store traffic; adds ~1e-3 abs error on
a 3.4 output scale).

Both token groups run the identical loop: for each leaf l and token
tile t, the two 512-col PSUM chains (psl/psr) interleave over the 8
k-chunks on a shared stationary xt tile, and for l==0 the 16-col
decision-logit matmul rides on the same stationary.  sigmoid+path
products run on ACT/DVE in the shadow of the next tile's chains, and
evictions scale by the path column (l==0 overwrites, l<3 accumulates
into acc, l==3 writes path-weighted result + acc into the bf16 output
tile).

DMA plan (the cold start is issue-rate/single-ring bound otherwise):
  sync HWDGE ring:  packed consts (wd+bd+ones, 1 issue), xt group-0
                    chunks, path exports, xt group-1 (mid-run), output
                    halves.
  scalar HWDGE ring: wl chunks, leaf 0 first, later leaves staged
                    mid-loop so their issue cost doesn't queue ahead of
                    the sigmoids on the ACT engine.
The last tile runs its two 512-col chains sequentially so the first
half's eviction+store drains under the second half's matmuls.
"""

import numpy as np

B, S, H = 2, 2048, 1024
DP, EP = 2, 4            # data-parallel x expert-parallel = 8 cores
T = (B * S) // DP        # 2048 tokens per core
LPC = 16 // EP           # 4 leaves per core
NT = T // 128            # 16 token tiles per core
TG = 2                   # token groups (acc working set = 8 tiles)
TPG = NT // TG           # 8 token tiles per group
KC = H // 128            # 8 contraction chunks
ND = 16                  # decision columns (14 used + 2 pad)
PKC = KC * ND + ND + 128  # packed const cols: wd | bd | ones

_prog_cache = {}


def _build_program():
    if "nc" in _prog_cache:
        return _prog_cache["nc"]

    from contextlib import ExitStack
    import concourse.bacc as bacc
    import concourse.tile as tile
    import concourse.mybir as mybir

    f32 = mybir.dt.float32
    f16 = mybir.dt.float16
    bf16 = mybir.dt.bfloat16
    MULT = mybir.AluOpType.mult
    ADD = mybir.AluOpType.add
    SIG = mybir.ActivationFunctionType.Sigmoid

    nc = bacc.Bacc("TRN2", target_bir_lowering=False, debug=False, num_devices=8)

    xt_d = nc.dram_tensor("xt", [H, T], f16, kind="ExternalInput").ap()
    wl_d = nc.dram_tensor("wl", [LPC, H, H], f16, kind="ExternalInput").ap()
    pk_d = nc.dram_tensor("pk", [128, PKC], f16, kind="ExternalInput").ap()
    out_d = nc.dram_tensor("out", [T, H], bf16, kind="ExternalOutput").ap()
    path_d = nc.dram_tensor("path", [TG, 128, TPG * LPC], f32,
                            kind="ExternalOutput").ap()

    with tile.TileContext(nc) as tc, ExitStack() as ctx:
        consts = ctx.enter_context(tc.tile_pool(name="consts", bufs=1))
        xt_pool = ctx.enter_context(tc.tile_pool(name="xt", bufs=1))
        wl_pool = ctx.enter_context(tc.tile_pool(name="wl", bufs=1))
        acc_pool = ctx.enter_context(tc.tile_pool(name="acc", bufs=1))
        dec_pool = ctx.enter_context(tc.tile_pool(name="dec", bufs=2))
        out_pool = ctx.enter_context(tc.tile_pool(name="osb", bufs=2))
        ps_pool = ctx.enter_context(tc.tile_pool(name="ps", bufs=8, space="PSUM"))

        # --- PE warmup: dep-free matmuls push HAM to 2.4GHz while the
        #     first DMAs land (~10 x 427ns cold covers the ~3.4us window
        #     plus the stretch until xt/wl chunks arrive) ---
        warm = consts.tile([128, 512], f16, tag="warm")
        nc.vector.memset(warm[:], 0.0)
        wps = ps_pool.tile([128, 512], f32, tag="ps", name="warmps")
        for i in range(10):
            nc.tensor.matmul(wps[:], warm[:, 0:128], warm[:],
                             start=True, stop=True)

        # --- packed consts: one issue on the sync ring, then xt g0 ---
        pk = consts.tile([128, PKC], f16, tag="pk")
        nc.sync.dma_start(pk[:], pk_d[:, :])
        wd_sb = pk[:, 0:KC * ND]
        bd_sb = pk[0:1, KC * ND:KC * ND + ND]
        ones = pk[0:1, KC * ND + ND:PKC]

        xt = {}

        def load_xt(g):
            for k in range(KC):
                t_ = xt_pool.tile([128, T // TG], f16, tag=f"xt{k}_{g}",
                                  name=f"xt{k}_{g}")
                nc.sync.dma_start(
                    t_[:], xt_d[k * 128:(k + 1) * 128,
                                g * (T // TG):(g + 1) * (T // TG)])
                xt[k, g] = t_

        load_xt(0)

        def lhsT_of(k, g, t):
            return xt[k, g][:, t * 128:(t + 1) * 128]

        wl_res = {}

        def load_wl(l):
            # scalar (ACT) HWDGE ring; staged per leaf so issue cost
            # never queues ahead of the sigmoids
            for k in range(KC):
                w = wl_pool.tile([128, H], f16, tag=f"wl{l}_{k}",
                                 name=f"wl{l}_{k}")
                nc.scalar.dma_start(w[:], wl_d[l, k * 128:(k + 1) * 128, :])
                wl_res[l, k] = w

        load_wl(0)

        # bd broadcast to all 128 partitions via ones-vector matmul
        bdb = consts.tile([128, ND], f32, tag="bdb")
        bp = ps_pool.tile([128, 512], f32, tag="ps", name="bdbps")
        nc.tensor.matmul(bp[:, 0:ND], ones, bd_sb, start=True, stop=True)
        nc.vector.tensor_copy(bdb[:], bp[:, 0:ND])

        for g in range(TG):
            dec_sb = dec_pool.tile([128, TPG * ND], f32, tag="dec")
            path = dec_pool.tile([128, TPG * LPC], f32, tag="path")
            accs = [acc_pool.tile([128, H], f32, tag=f"acc{t}",
                                  name=f"acc{t}_{g}")
                    for t in range(TPG)]
            outs = [out_pool.tile([128, H], bf16, tag=f"osb{t}",
                                  name=f"osb{t}_{g}")
                    for t in range(TPG)]

            def sig_path(t, dps):
                # sigmoid(dec + bd), then this tile's 4 path columns
                tadd = dec_pool.tile([128, ND], f32, tag="tadd",
                                     name=f"tadd{t}_{g}")
                nc.vector.tensor_tensor(tadd[:], dps, bdb[:], op=ADD)
                dsl = dec_sb[:, t * ND:(t + 1) * ND]
                nc.scalar.activation(dsl, tadd[:], SIG)
                d3 = dsl.rearrange("p (n c) -> p n c", c=2)
                pt = path[:, t * LPC:(t + 1) * LPC]
                # P4[m] = P2[m%2] * dec[node 1+m%2, choice m//2]
                p4 = dec_pool.tile([128, 4], f32, tag="p4",
                                   name=f"p4_{t}_{g}")
                nc.vector.tensor_tensor(
                    p4[:, 0:2], dsl[:, 0:2], d3[:, 1:3, 0], op=MULT)
                nc.vector.tensor_tensor(
                    p4[:, 2:4], dsl[:, 0:2], d3[:, 1:3, 1], op=MULT)
                p4b = dec_pool.tile([128, 4], f32, tag="p4b",
                                    name=f"p4b_{t}_{g}")
                nc.vector.tensor_tensor(p4b[:], p4[:], dsl[:, 6:10], op=MULT)
                nc.vector.tensor_tensor(pt, p4b[:], dsl[:, 10:14], op=MULT)

            def evict(t, l, ps_t, half):
                # acc/out update with the path-scaled psum chain
                pcol = path[:, t * LPC + l:t * LPC + l + 1]
                o = half * 512
                if l == 0:
                    nc.vector.tensor_scalar(
                        accs[t][:, o:o + 512], ps_t[:], pcol, None, op0=MULT)
                elif l < LPC - 1:
                    nc.vector.scalar_tensor_tensor(
                        accs[t][:, o:o + 512], ps_t[:], pcol,
                        accs[t][:, o:o + 512], op0=MULT, op1=ADD)
                else:
                    nc.vector.scalar_tensor_tensor(
                        outs[t][:, o:o + 512], ps_t[:], pcol,
                        accs[t][:, o:o + 512], op0=MULT, op1=ADD)

            for l in range(LPC):
                wls = [wl_res[l, k] for k in range(KC)]
                for t in range(TPG):
                    if g == 0 and l == 0 and t == 0:
                        # cold-start pairing: process tiles 0+1 k-outer so
                        # each arriving (xt, wl) chunk supplies ~0.9us of
                        # real PE work — keeps HAM busy without idling on
                        # chunk latency.  6 concurrent PSUM chains.
                        pls, prs, ds = [], [], []
                        for tt in (0, 1):
                            pls.append(ps_pool.tile([128, 512], f32,
                                                    tag="ps", name=f"Pl{tt}"))
                            prs.append(ps_pool.tile([128, 512], f32,
                                                    tag="ps", name=f"Pr{tt}"))
                            ds.append(ps_pool.tile([128, 512], f32,
                                                   tag="ps", name=f"Pd{tt}"))
                        for k in range(KC):
                            for tt in (0, 1):
                                lhsT = lhsT_of(k, g, tt)
                                nc.tensor.matmul(
                                    pls[tt][:], lhsT, wls[k][:, 0:512],
                                    start=(k == 0), stop=(k == KC - 1))
                                nc.tensor.matmul(
                                    prs[tt][:], lhsT, wls[k][:, 512:1024],
                                    start=(k == 0), stop=(k == KC - 1))
                                nc.tensor.matmul(
                                    ds[tt][:, 0:ND], lhsT,
                                    wd_sb[:, k * ND:(k + 1) * ND],
                                    start=(k == 0), stop=(k == KC - 1))
                            if k >= 1:
                                nc.tensor.matmul(wps[:], warm[:, 0:128],
                                                 warm[:],
                                                 start=True, stop=True)
                        for tt in (0, 1):
                            sig_path(tt, ds[tt][:, 0:ND])
                            evict(tt, l, pls[tt], 0)
                            evict(tt, l, prs[tt], 1)
                        continue
                    if g == 0 and l == 0 and t == 1:
                        load_wl(1)
                        continue
                    last = (g == TG - 1 and l == LPC - 1 and t == TPG - 1)
                    r0 = (g * TPG + t) * 128
                    psl = ps_pool.tile([128, 512], f32, tag="ps",
                                       name=f"pl{l}_{t}_{g}")
                    psr = ps_pool.tile([128, 512], f32, tag="ps",
                                       name=f"pr{l}_{t}_{g}")
                    if last:
                        # half then quarters: each eviction+store drains
                        # under the next chain's matmuls, leaving a 64KB
                        # store as the only post-matmul tail
                        for k in range(KC):
                            nc.tensor.matmul(
                                psl[:], lhsT_of(k, g, t), wls[k][:, 0:512],
                                start=(k == 0), stop=(k == KC - 1))
                        evict(t, l, psl, 0)
                        nc.scalar.dma_start(out_d[r0:r0 + 128, 0:512],
                                            outs[t][:, 0:512])
                        pcol = path[:, t * LPC + l:t * LPC + l + 1]
                        for q in range(2):
                            c0 = 512 + q * 256
                            pq = psr if q == 0 else ps_pool.tile(
                                [128, 512], f32, tag="ps", name=f"pq{t}_{g}")
                            for k in range(KC):
                                nc.tensor.matmul(
                                    pq[:, 0:256],
                                    lhsT_of(k, g, t), wls[k][:, c0:c0 + 256],
                                    start=(k == 0), stop=(k == KC - 1))
                            nc.vector.scalar_tensor_tensor(
                                outs[t][:, c0:c0 + 256], pq[:, 0:256], pcol,
                                accs[t][:, c0:c0 + 256], op0=MULT, op1=ADD)
                            nc.scalar.dma_start(
                                out_d[r0:r0 + 128, c0:c0 + 256],
                                outs[t][:, c0:c0 + 256])
                        continue
                    dps = None
                    if l == 0:
                        dps = ps_pool.tile([128, 512], f32, tag="ps",
                                           name=f"dp{t}_{g}")
                    for k in range(KC):
                        lhsT = lhsT_of(k, g, t)
                        nc.tensor.matmul(psl[:], lhsT, wls[k][:, 0:512],
                                         start=(k == 0), stop=(k == KC - 1))
                        nc.tensor.matmul(psr[:], lhsT, wls[k][:, 512:1024],
                                         start=(k == 0), stop=(k == KC - 1))
                        if dps is not None:
                            # decision logits ride on the same stationary
                            nc.tensor.matmul(
                                dps[:, 0:ND], lhsT,
                                wd_sb[:, k * ND:(k + 1) * ND],
                                start=(k == 0), stop=(k == KC - 1))
                    if dps is not None:
                        sig_path(t, dps[:, 0:ND])
                    evict(t, l, psl, 0)
                    evict(t, l, psr, 1)
                    if l == LPC - 1:
                        # group-1 stores ride the scalar ring (idle after
                        # the wl loads) so the tail never queues behind
                        # the sync-ring FIFO
                        ring = nc.scalar if g == TG - 1 else nc.sync
                        ring.dma_start(out_d[r0:r0 + 128, 0:512],
                                       outs[t][:, 0:512])
                        ring.dma_start(out_d[r0:r0 + 128, 512:1024],
                                       outs[t][:, 512:1024])
                    # staged loads / exports
                    if g == 0 and l == 0 and t == 4:
                        load_wl(2)
                    if g == 0 and l == 0 and t == TPG - 1:
                        nc.sync.dma_start(path_d[0], path[:])
                    if g == 0 and l == 1 and t == 0:
                        load_wl(3)
                        load_xt(1)
                    if g == 1 and l == 0 and t == TPG - 1:
                        nc.sync.dma_start(path_d[1], path[:])

    nc.compile()
    _prog_cache["nc"] = nc
    return nc


def _core_inputs(x, Wd, bd, Wl, bl):
    """Build the 8 per-core input dicts (host-side sharding)."""
    x2 = np.ascontiguousarray(x, dtype=np.float32).reshape(B * S, H)
    Wd = np.asarray(Wd, dtype=np.float32)
    bd = np.asarray(bd, dtype=np.float32)
    Wl = np.ascontiguousarray(Wl, dtype=np.float32)

    xts = [np.ascontiguousarray(x2[d * T:(d + 1) * T].T).astype(np.float16)
           for d in range(DP)]

    in_maps = []
    for c in range(8):
        d, e = c // EP, c % EP
        e1, e0 = e // 2, e % 2
        # per-core decision matrix: cols 0..5 nodes 0,1,2 (both choices),
        # 6..9 level-2 factor per leaf, 10..13 level-3 factor per leaf
        wd_c = np.zeros((H, ND), dtype=np.float32)
        bd_c = np.zeros(ND, dtype=np.float32)
        for n in range(3):
            wd_c[:, 2 * n:2 * n + 2] = Wd[n]
            bd_c[2 * n:2 * n + 2] = bd[n]
        for l in range(4):
            wd_c[:, 6 + l] = Wd[3 + l, :, e0]
            bd_c[6 + l] = bd[3 + l, e0]
            n3 = 7 + 4 * e0 + l
            wd_c[:, 10 + l] = Wd[n3, :, e1]
            bd_c[10 + l] = bd[n3, e1]
        pk = np.zeros((128, PKC), dtype=np.float16)
        # wd chunk layout: pk[p, k*ND+n] = wd_c[k*128+p, n]
        pk[:, 0:KC * ND] = (
            wd_c.reshape(KC, 128, ND).transpose(1, 0, 2).reshape(128, KC * ND))
        pk[0, KC * ND:KC * ND + ND] = bd_c
        pk[0, KC * ND + ND:PKC] = 1.0
        in_maps.append({
            "xt": xts[d],
            "wl": np.ascontiguousarray(
                Wl[LPC * e:LPC * (e + 1)]).astype(np.float16),
            "pk": pk,
        })
    return in_maps


def kernel(x, Wd, bd, Wl, bl, _want_results=False):
    from concourse import bass_utils

    nc = _build_program()
    in_maps = _core_inputs(x, Wd, bd, Wl, bl)
    res = bass_utils.run_bass_kernel_spmd(nc, in_maps, list(range(8)))

    bl64 = np.asarray(bl, dtype=np.float64)
    out = np.empty((DP, T, H), dtype=np.float32)
    for d in range(DP):
        s = np.zeros((T, H), dtype=np.float64)
        for e in range(EP):
            r = res.results[d * EP + e]
            s += np.asarray(r["out"], dtype=np.float64)
            # path export [TG, 128, TPG*LPC] -> [T, LPC]; token index is
            # g*1024 + t*128 + p
            p = np.asarray(r["path"], dtype=np.float64)
            p = p.reshape(TG, 128, TPG, LPC).transpose(0, 2, 1, 3)
            p = p.reshape(T, LPC)
            s += p @ bl64[LPC * e:LPC * (e + 1)]
        out[d] = s.astype(np.float32)
    out = out.reshape(B, S, H)
    if _want_results:
        return out, res
    return out


# revision 12
# speedup vs baseline: 1.1849x; 1.1849x over previous
"""Soft decision-tree layer (depth 4, 16 leaves) on 8 trn2 NeuronCores.

Sharding: 2-way data parallel (token halves) x 4-way expert parallel
(4 leaves per core).  Each core computes, for its 2048 tokens t and its
4 leaves l:  partial[t,:] = sum_l path_l(t) * (x[t] @ Wl[l]), plus it
exports the path probabilities; the host adds the path-weighted leaf
biases (sum_l path_l * bl[l]) in float64 and sums the 4 expert partials
per token half.

GEMM operands are float16 (streams at the full 1 col/cycle PE rate with
the weight load pipelined under the previous matmul; ~216ns per
128x128x512 matmul); accumulation is fp32 in PSUM/SBUF; partials are
written back as bf16 (halves the store traffic; adds ~1e-3 abs error on
a 3.4 output scale).

Both token groups run the identical loop: for each leaf l and token
tile t, the two 512-col PSUM chains (psl/psr) interleave over the 8
k-chunks on a shared stationary xt tile, and for l==0 the 16-col
decision-logit matmul rides on the same stationary.  sigmoid+path
products run on ACT/DVE in the shadow of the next tile's chains, and
evictions scale by the path column (l==0 overwrites, l<3 accumulates
into acc, l==3 writes path-weighted result + acc into the bf16 output
tile).

DMA plan (the cold start is issue-rate/single-ring bound otherwise):
  sync HWDGE ring:  packed consts (wd+bd+ones, 1 issue), xt group-0
                    chunks, path exports, xt group-1 (mid-run), output
                    halves.
  scalar HWDGE ring: wl chunks, leaf 0 first, later leaves staged
                    mid-loop so their issue cost doesn't queue ahead of
                    the sigmoids on the ACT engine.
The last tile runs its two 512-col chains sequentially so the first
half's eviction+store drains under the second half's matmuls.
"""

import numpy as np

B, S, H = 2, 2048, 1024
DP, EP = 2, 4            # data-parallel x expert-parallel = 8 cores
T = (B * S) // DP        # 2048 tokens per core
LPC = 16 // EP           # 4 leaves per core
NT = T // 128            # 16 token tiles per core
TG = 2                   # token groups (acc working set = 8 tiles)
TPG = NT // TG           # 8 token tiles per group
KC = H // 128            # 8 contraction chunks
ND = 16                  # decision columns (14 used + 2 pad)
PKC = KC * ND + ND + 128  # packed const cols: wd | bd | ones

_prog_cache = {}


def _build_program():
    if "nc" in _prog_cache:
        return _prog_cache["nc"]

    from contextlib import ExitStack
    import concourse.bacc as bacc
    import concourse.tile as tile
    import concourse.mybir as mybir

    f32 = mybir.dt.float32
    f16 = mybir.dt.float16
    bf16 = mybir.dt.bfloat16
    MULT = mybir.AluOpType.mult
    ADD = mybir.AluOpType.add
    SIG = mybir.ActivationFunctionType.Sigmoid

    nc = bacc.Bacc("TRN2", target_bir_lowering=False, debug=False, num_devices=8)

    xt_d = nc.dram_tensor("xt", [H, T], f16, kind="ExternalInput").ap()
    wl_d = nc.dram_tensor("wl", [LPC, H, H], f16, kind="ExternalInput").ap()
    pk_d = nc.dram_tensor("pk", [128, PKC], f16, kind="ExternalInput").ap()
    out_d = nc.dram_tensor("out", [T, H], bf16, kind="ExternalOutput").ap()
    path_d = nc.dram_tensor("path", [TG, 128, TPG * LPC], f32,
                            kind="ExternalOutput").ap()

    with tile.TileContext(nc) as tc, ExitStack() as ctx:
        consts = ctx.enter_context(tc.tile_pool(name="consts", bufs=1))
        xt_pool = ctx.enter_context(tc.tile_pool(name="xt", bufs=1))
        wl_pool = ctx.enter_context(tc.tile_pool(name="wl", bufs=1))
        acc_pool = ctx.enter_context(tc.tile_pool(name="acc", bufs=1))
        dec_pool = ctx.enter_context(tc.tile_pool(name="dec", bufs=2))
        out_pool = ctx.enter_context(tc.tile_pool(name="osb", bufs=2))
        ps_pool = ctx.enter_context(tc.tile_pool(name="ps", bufs=7, space="PSUM"))
        wf_pool = ctx.enter_context(tc.tile_pool(name="wf", bufs=1, space="PSUM"))

        # --- PE warmup: dep-free matmuls push HAM to 2.4GHz while the
        #     first DMAs land (~10 x 427ns cold covers the ~3.4us window
        #     plus the stretch until xt/wl chunks arrive) ---
        warm = consts.tile([128, 512], f16, tag="warm")
        nc.vector.memset(warm[:], 0.0)
        wps = wf_pool.tile([128, 512], f32, tag="wf", name="warmps")
        for i in range(10):
            nc.tensor.matmul(wps[:], warm[:, 0:128], warm[:],
                             start=True, stop=True)

        # --- packed consts: one issue on the sync ring, then xt g0 ---
        pk = consts.tile([128, PKC], f16, tag="pk")
        nc.sync.dma_start(pk[:], pk_d[:, :])
        wd_sb = pk[:, 0:KC * ND]
        bd_sb = pk[0:1, KC * ND:KC * ND + ND]
        ones = pk[0:1, KC * ND + ND:PKC]

        xt = {}

        def load_xt(g):
            for k in range(KC):
                t_ = xt_pool.tile([128, T // TG], f16, tag=f"xt{k}_{g}",
                                  name=f"xt{k}_{g}")
                nc.sync.dma_start(
                    t_[:], xt_d[k * 128:(k + 1) * 128,
                                g * (T // TG):(g + 1) * (T // TG)])
                xt[k, g] = t_

        load_xt(0)

        def lhsT_of(k, g, t):
            return xt[k, g][:, t * 128:(t + 1) * 128]

        wl_res = {}

        def load_wl(l):
            # scalar (ACT) HWDGE ring; staged per leaf so issue cost
            # never queues ahead of the sigmoids
            for k in range(KC):
                w = wl_pool.tile([128, H], f16, tag=f"wl{l}_{k}",
                                 name=f"wl{l}_{k}")
                nc.scalar.dma_start(w[:], wl_d[l, k * 128:(k + 1) * 128, :])
                wl_res[l, k] = w

        load_wl(0)

        # bd broadcast to all 128 partitions via ones-vector matmul
        bdb = consts.tile([128, ND], f32, tag="bdb")
        bp = ps_pool.tile([128, 512], f32, tag="ps", name="bdbps")
        nc.tensor.matmul(bp[:, 0:ND], ones, bd_sb, start=True, stop=True)
        nc.vector.tensor_copy(bdb[:], bp[:, 0:ND])

        for g in range(TG):
            dec_sb = dec_pool.tile([128, TPG * ND], f32, tag="dec")
            path = dec_pool.tile([128, TPG * LPC], f32, tag="path")
            accs = [acc_pool.tile([128, H], f32, tag=f"acc{t}",
                                  name=f"acc{t}_{g}")
                    for t in range(TPG)]
            outs = [out_pool.tile([128, H], bf16, tag=f"osb{t}",
                                  name=f"osb{t}_{g}")
                    for t in range(TPG)]

            def sig_path(t, dps):
                # sigmoid(dec + bd), then this tile's 4 path columns
                tadd = dec_pool.tile([128, ND], f32, tag="tadd",
                                     name=f"tadd{t}_{g}")
                nc.vector.tensor_tensor(tadd[:], dps, bdb[:], op=ADD)
                dsl = dec_sb[:, t * ND:(t + 1) * ND]
                nc.scalar.activation(dsl, tadd[:], SIG)
                d3 = dsl.rearrange("p (n c) -> p n c", c=2)
                pt = path[:, t * LPC:(t + 1) * LPC]
                # P4[m] = P2[m%2] * dec[node 1+m%2, choice m//2]
                p4 = dec_pool.tile([128, 4], f32, tag="p4",
                                   name=f"p4_{t}_{g}")
                nc.vector.tensor_tensor(
                    p4[:, 0:2], dsl[:, 0:2], d3[:, 1:3, 0], op=MULT)
                nc.vector.tensor_tensor(
                    p4[:, 2:4], dsl[:, 0:2], d3[:, 1:3, 1], op=MULT)
                p4b = dec_pool.tile([128, 4], f32, tag="p4b",
                                    name=f"p4b_{t}_{g}")
                nc.vector.tensor_tensor(p4b[:], p4[:], dsl[:, 6:10], op=MULT)
                nc.vector.tensor_tensor(pt, p4b[:], dsl[:, 10:14], op=MULT)

            def evict(t, l, ps_t, half):
                # acc/out update with the path-scaled psum chain
                pcol = path[:, t * LPC + l:t * LPC + l + 1]
                o = half * 512
                if l == 0:
                    nc.vector.tensor_scalar(
                        accs[t][:, o:o + 512], ps_t[:], pcol, None, op0=MULT)
                elif l < LPC - 1:
                    nc.vector.scalar_tensor_tensor(
                        accs[t][:, o:o + 512], ps_t[:], pcol,
                        accs[t][:, o:o + 512], op0=MULT, op1=ADD)
                else:
                    nc.vector.scalar_tensor_tensor(
                        outs[t][:, o:o + 512], ps_t[:], pcol,
                        accs[t][:, o:o + 512], op0=MULT, op1=ADD)

            for l in range(LPC):
                wls = [wl_res[l, k] for k in range(KC)]
                for t in range(TPG):
                    if g == 0 and l == 0 and t == 0:
                        # cold-start pairing: process tiles 0+1 k-outer so
                        # each arriving (xt, wl) chunk supplies ~0.9us of
                        # real PE work — keeps HAM busy without idling on
                        # chunk latency.  6 concurrent PSUM chains.
                        pls, prs, ds = [], [], []
                        for tt in (0, 1):
                            pls.append(ps_pool.tile([128, 512], f32,
                                                    tag="ps", name=f"Pl{tt}"))
                            prs.append(ps_pool.tile([128, 512], f32,
                                                    tag="ps", name=f"Pr{tt}"))
                            ds.append(ps_pool.tile([128, 512], f32,
                                                   tag="ps", name=f"Pd{tt}"))
                        for k in range(KC):
                            for tt in (0, 1):
                                lhsT = lhsT_of(k, g, tt)
                                nc.tensor.matmul(
                                    pls[tt][:], lhsT, wls[k][:, 0:512],
                                    start=(k == 0), stop=(k == KC - 1))
                                nc.tensor.matmul(
                                    prs[tt][:], lhsT, wls[k][:, 512:1024],
                                    start=(k == 0), stop=(k == KC - 1))
                                nc.tensor.matmul(
                                    ds[tt][:, 0:ND], lhsT,
                                    wd_sb[:, k * ND:(k + 1) * ND],
                                    start=(k == 0), stop=(k == KC - 1))
                            if k >= 1:
                                nc.tensor.matmul(wps[:], warm[:, 0:128],
                                                 warm[:],
                                                 start=True, stop=True)
                        for tt in (0, 1):
                            sig_path(tt, ds[tt][:, 0:ND])
                            evict(tt, l, pls[tt], 0)
                            evict(tt, l, prs[tt], 1)
                        continue
                    if g == 0 and l == 0 and t == 1:
                        load_wl(1)
                        continue
                    if g == 0 and l == 0 and t in (2, 3):
                        for _ in range(2):
                            nc.tensor.matmul(wps[:], warm[:, 0:128], warm[:],
                                             start=True, stop=True)
                    last = (g == TG - 1 and l == LPC - 1 and t == TPG - 1)
                    r0 = (g * TPG + t) * 128
                    psl = ps_pool.tile([128, 512], f32, tag="ps",
                                       name=f"pl{l}_{t}_{g}")
                    psr = ps_pool.tile([128, 512], f32, tag="ps",
                                       name=f"pr{l}_{t}_{g}")
                    if last:
                        # half then quarters: each eviction+store drains
                        # under the next chain's matmuls, leaving a 64KB
                        # store as the only post-matmul tail
                        for k in range(KC):
                            nc.tensor.matmul(
                                psl[:], lhsT_of(k, g, t), wls[k][:, 0:512],
                                start=(k == 0), stop=(k == KC - 1))
                        evict(t, l, psl, 0)
                        nc.scalar.dma_start(out_d[r0:r0 + 128, 0:512],
                                            outs[t][:, 0:512])
                        pcol = path[:, t * LPC + l:t * LPC + l + 1]
                        for q in range(2):
                            c0 = 512 + q * 256
                            pq = psr if q == 0 else ps_pool.tile(
                                [128, 512], f32, tag="ps", name=f"pq{t}_{g}")
                            for k in range(KC):
                                nc.tensor.matmul(
                                    pq[:, 0:256],
                                    lhsT_of(k, g, t), wls[k][:, c0:c0 + 256],
                                    start=(k == 0), stop=(k == KC - 1))
                            nc.vector.scalar_tensor_tensor(
                                outs[t][:, c0:c0 + 256], pq[:, 0:256], pcol,
                                accs[t][:, c0:c0 + 256], op0=MULT, op1=ADD)
                            nc.scalar.dma_start(
                                out_d[r0:r0 + 128, c0:c0 + 256],
                                outs[t][:, c0:c0 + 256])
                        continue
                    dps = None
                    if l == 0:
                        dps = ps_pool.tile([128, 512], f32, tag="ps",
                                           name=f"dp{t}_{g}")
                    for k in range(KC):
                        lhsT = lhsT_of(k, g, t)
                        nc.tensor.matmul(psl[:], lhsT, wls[k][:, 0:512],
                                         start=(k == 0), stop=(k == KC - 1))
                        nc.tensor.matmul(psr[:], lhsT, wls[k][:, 512:1024],
                                         start=(k == 0), stop=(k == KC - 1))
                        if dps is not None:
                            # decision logits ride on the same stationary
                            nc.tensor.matmul(
                                dps[:, 0:ND], lhsT,
                                wd_sb[:, k * ND:(k + 1) * ND],
                                start=(k == 0), stop=(k == KC - 1))
                    if dps is not None:
                        sig_path(t, dps[:, 0:ND])
                    evict(t, l, psl, 0)
                    evict(t, l, psr, 1)
                    if l == LPC - 1:
                        # group-1 stores ride the scalar ring (idle after
                        # the wl loads) so the tail never queues behind
                        # the sync-ring FIFO
                        ring = nc.scalar if g == TG - 1 else nc.sync
                        ring.dma_start(out_d[r0:r0 + 128, 0:512],
                                       outs[t][:, 0:512])
                        ring.dma_start(out_d[r0:r0 + 128, 512:1024],
                                       outs[t][:, 512:1024])
                    # staged loads / exports
                    if g == 0 and l == 0 and t == 4:
                        load_wl(2)
                    if g == 0 and l == 0 and t == TPG - 1:
                        nc.sync.dma_start(path_d[0], path[:])
                    if g == 0 and l == 1 and t == 0:
                        load_wl(3)
                        load_xt(1)
                    if g == 1 and l == 0 and t == TPG - 1:
                        nc.sync.dma_start(path_d[1], path[:])

    nc.compile()
    _prog_cache["nc"] = nc
    return nc


def _core_inputs(x, Wd, bd, Wl, bl):
    """Build the 8 per-core input dicts (host-side sharding)."""
    x2 = np.ascontiguousarray(x, dtype=np.float32).reshape(B * S, H)
    Wd = np.asarray(Wd, dtype=np.float32)
    bd = np.asarray(bd, dtype=np.float32)
    Wl = np.ascontiguousarray(Wl, dtype=np.float32)

    xts = [np.ascontiguousarray(x2[d * T:(d + 1) * T].T).astype(np.float16)
           for d in range(DP)]

    in_maps = []
    for c in range(8):
        d, e = c // EP, c % EP
        e1, e0 = e // 2, e % 2
        # per-core decision matrix: cols 0..5 nodes 0,1,2 (both choices),
        # 6..9 level-2 factor per leaf, 10..13 level-3 factor per leaf
        wd_c = np.zeros((H, ND), dtype=np.float32)
        bd_c = np.zeros(ND, dtype=np.float32)
        for n in range(3):
            wd_c[:, 2 * n:2 * n + 2] = Wd[n]
            bd_c[2 * n:2 * n + 2] = bd[n]
        for l in range(4):
            wd_c[:, 6 + l] = Wd[3 + l, :, e0]
            bd_c[6 + l] = bd[3 + l, e0]
            n3 = 7 + 4 * e0 + l
            wd_c[:, 10 + l] = Wd[n3, :, e1]
            bd_c[10 + l] = bd[n3, e1]
        pk = np.zeros((128, PKC), dtype=np.float16)
        # wd chunk layout: pk[p, k*ND+n] = wd_c[k*128+p, n]
        pk[:, 0:KC * ND] = (
            wd_c.reshape(KC, 128, ND).transpose(1, 0, 2).reshape(128, KC * ND))
        pk[0, KC * ND:KC * ND + ND] = bd_c
        pk[0, KC * ND + ND:PKC] = 1.0
        in_maps.append({
            "xt": xts[d],
            "wl": np.ascontiguousarray(
                Wl[LPC * e:LPC * (e + 1)]).astype(np.float16),
            "pk": pk,
        })
    return in_maps


def kernel(x, Wd, bd, Wl, bl, _want_results=False):
    from concourse import bass_utils

    nc = _build_program()
    in_maps = _core_inputs(x, Wd, bd, Wl, bl)
    res = bass_utils.run_bass_kernel_spmd(nc, in_maps, list(range(8)))

    bl64 = np.asarray(bl, dtype=np.float64)
    out = np.empty((DP, T, H), dtype=np.float32)
    for d in range(DP):
        s = np.zeros((T, H), dtype=np.float64)
        for e in range(EP):
            r = res.results[d * EP + e]
            s += np.asarray(r["out"], dtype=np.float64)
            # path export [TG, 128, TPG*LPC] -> [T, LPC]; token index is
            # g*1024 + t*128 + p
            p = np.asarray(r["path"], dtype=np.float64)
            p = p.reshape(TG, 128, TPG, LPC).transpose(0, 2, 1, 3)
            p = p.reshape(T, LPC)
            s += p @ bl64[LPC * e:LPC * (e + 1)]
        out[d] = s.astype(np.float32)
    out = out.reshape(B, S, H)
    if _want_results:
        return out, res
    return out
